# revision 2
# baseline (speedup 1.0000x reference)
"""Trainium2 Bass kernel for nn_EpsiLayer.

Per-channel causal full-length time conv
  out[b,t,j] = P[b,t,j] + sum_{k<=t} g[k,j] * P[b,t-k,j]
with g'[0] = g[0]+1 identity fold, computed as blocked lower-tri
Toeplitz matmuls y_i += W_d @ x_{i-d} (C=128 blocks, Hankel tiles
W_d[p,a] = gpad[d*128+a+p]).  Channel-parallel: 32 channels/core on 8
cores, zero inter-core communication.

HW model (measured): per-MM ~ 30ns fixed + 0.41ns/col regardless of
weight reuse or FWL, so conv floor = 32ch x (32 MMs x 30ns + 4224
cols x 0.41ns) ~ 86us/core; HBM stream ~ 34MB / 358GB/s ~ 96us.
The kernel is DMA-paced; measured wall ~ 115-125us (run-to-run HBM
variance +-5us).

Structure:
  - NGEN=4 tail tiles per channel are PE-generated from compact seeds
    (4 shift-matrix matmuls per channel, each one wide-N contiguous
    PSUM region per z; DVE does the stride-4 interleave into SBUF via
    a stepped dst AP).  Larger NGEN is PE-cost negative: gen MMs pay
    the same 30ns/MM fixed cost.
  - first DPRE=4 channels are pure-dense (no gen deps) so conv ch0
    starts as soon as its first 8 weight tiles land.
  - weight DMA split head(8 tiles)/tail per channel on the sync HWDGE
    ring, x tiles interleaved in channel order just-in-time (x leads
    hurt: weights-first priority wins).  Finer piece-splitting of the
    weight DMAs regresses badly (per-DMA completion overhead).
  - out DMAs issued from sync; og copies alternate scalar/vector; gen
    interleave copies on vector.  PSUM: 5 acc banks + 3 gen banks.
  - tail: last channels flush out per-channel.
"""

import sys
import numpy as np

try:
    from concourse import bacc, tile  # noqa: F401
except ImportError:
    sys.path.insert(0, "/opt/trn_rl_repo")

import ml_dtypes

B, T, NR = 8, 4096, 256
C = 128
NB = T // C            # 32 time blocks
N_CORES = 8
CPC = NR // N_CORES    # 32 channels per core
COLS = CPC * NB * B    # 8192 columns per core
GLEN = 127 + T + 1     # 4224

_cache = {}


def _build_nc(OB=2, XSPLIT=16, oeng="scalar", wbufs=10, pbufs=4, obufs=4,
              obf16=True, ngen=4, nz=4, gbufs=3, gpbufs=4, geng="alt",
              sqeng="gpsimd", look=3, dpre=4, hsplit=8, warmup=0, xlead=2,
              obtail=1, xeng="sync", oceng="alt"):
    from concourse import bacc, tile
    import concourse.mybir as mybir

    NZ = nz
    SEEDY = C // NZ
    GROWS = C - (NZ - 1)
    NGEN = ngen
    ND = NB - NGEN            # dense offsets for gen channels
    DPRE = dpre if NGEN else 0
    NGC = CPC - DPRE          # number of gen channels
    assert NGC % 2 == 0
    ZREG = NGEN * SEEDY       # cols per z-region in gen PSUM
    # z-regions packed into 2KB PSUM bank tiles (512 fp32 cols)
    ZPB = max(1, 512 // ZREG)         # z-regions per bank tile
    NBT = (NZ + ZPB - 1) // ZPB       # bank tiles per channel gen

    nc = bacc.Bacc("TRN2", target_bir_lowering=False, debug=False)

    HS = hsplit
    PS = hsplit                       # piece size in tiles
    WCOLS = ND * C + NGEN * SEEDY     # gen-channel weight cols
    # dense prologue channels: full NB dense tiles
    wpre_d = nc.dram_tensor("wpre", [DPRE, C, NB * C], mybir.dt.bfloat16,
                            kind="ExternalInput") if DPRE else None
    w_d = nc.dram_tensor("wdense", [NGC, C, WCOLS], mybir.dt.bfloat16,
                         kind="ExternalInput")
    x_d = nc.dram_tensor("xmov", [XSPLIT, C, COLS // XSPLIT],
                         mybir.dt.bfloat16, kind="ExternalInput")
    if NGEN:
        strip_d = nc.dram_tensor("strips", [NGC // 2, NZ - 1, 2 * NGEN * C],
                                 mybir.dt.bfloat16, kind="ExternalInput")
        shift_d = nc.dram_tensor("shifts", [C, NZ * C], mybir.dt.bfloat16,
                                 kind="ExternalInput")
    odt = mybir.dt.bfloat16 if obf16 else mybir.dt.float32
    o_d = nc.dram_tensor("out", [C, COLS], odt, kind="ExternalOutput")

    def _copy(eng, dst, src):
        if eng == "scalar":
            nc.scalar.activation(dst, src, mybir.ActivationFunctionType.Copy)
        else:
            getattr(nc, eng).tensor_copy(dst, src)

    XCH = COLS // XSPLIT        # columns per x tile
    CHX = CPC // XSPLIT         # channels per x tile

    with tile.TileContext(nc) as tc:
        with (
            tc.tile_pool(name="xpool", bufs=1) as xpool,
            tc.tile_pool(name="whpool", bufs=wbufs) as whpool,
            tc.tile_pool(name="wtpool", bufs=wbufs) as wtpool,
            tc.tile_pool(name="wppool", bufs=min(dpre + 1, 4)) as wppool,
            tc.tile_pool(name="opool", bufs=obufs) as opool,
            tc.tile_pool(name="gwpool", bufs=gbufs) as gwpool,
            tc.tile_pool(name="psum", bufs=pbufs, space="PSUM") as psum,
            tc.tile_pool(name="gpsum", bufs=gpbufs, space="PSUM") as gpsum,
        ):
            xts = [xpool.tile([C, XCH], mybir.dt.bfloat16, name=f"x{k}",
                              tag=f"x{k}") for k in range(XSPLIT)]
            shifts = None
            if warmup:
                wu = xpool.tile([C, C], mybir.dt.bfloat16, tag="warm")
                wup = gpsum.tile([C, 512], mybir.dt.float32, tag="pg")
                nc.vector.memset(wu[:], 0)
                for _ in range(warmup):
                    nc.tensor.matmul(wup[:, 0:C], wu[:], wu[:],
                                     start=True, stop=True)

            wpieces = {}

            def issue_w(j):
                """Issue channel j's weight DMA (head piece + tail)."""
                NPRE = min(1 + (xlead + CHX - 1) // CHX, XSPLIT)
                xe = getattr(nc, xeng)
                if j == 0:
                    for k0 in range(NPRE):
                        (nc.sync if k0 == 0 else xe).dma_start(
                            xts[k0][:], x_d[k0])
                jx = j + xlead
                if jx % CHX == 0 and NPRE <= jx // CHX < XSPLIT:
                    k = jx // CHX
                    xe.dma_start(xts[k][:], x_d[k])
                if j < DPRE:
                    wh = whpool.tile([C, HS * C], mybir.dt.bfloat16,
                                     tag="wh")
                    wt = wppool.tile([C, (NB - HS) * C], mybir.dt.bfloat16,
                                     tag="wtp")
                    nc.sync.dma_start(wh[:], wpre_d[j, :, 0:HS * C])
                    nc.sync.dma_start(wt[:], wpre_d[j, :, HS * C:])
                else:
                    wh = whpool.tile([C, HS * C], mybir.dt.bfloat16,
                                     tag="wh")
                    wt = wtpool.tile([C, WCOLS - HS * C], mybir.dt.bfloat16,
                                     tag="wt")
                    nc.sync.dma_start(wh[:], w_d[j - DPRE, :, 0:HS * C])
                    nc.sync.dma_start(wt[:], w_d[j - DPRE, :, HS * C:])
                wpieces[j] = [wh, wt]

            pair_tiles = {}

            def gen_tiles(j):
                """PE-generate channel j's tail weight tiles (d >= ND)."""
                nonlocal shifts
                if shifts is None:
                    shifts = xpool.tile([C, NZ * C], mybir.dt.bfloat16,
                                        tag="shifts")
                    nc.scalar.dma_start(shifts[:], shift_d.ap())
                jg = j - DPRE
                sj = wpieces[j][-1]
                soff = (ND - HS) * C
                if jg % 2 == 0:
                    wgp = gwpool.tile([C, 2 * NGEN * C], mybir.dt.bfloat16,
                                      tag="wg")
                    getattr(nc, sqeng).dma_start(wgp[GROWS:C, :],
                                                 strip_d[jg // 2])
                    pair_tiles[j + 1] = wgp
                else:
                    wgp = pair_tiles.pop(j)
                woff = (jg % 2) * NGEN * C
                pga = gpsum.tile([C, 512], mybir.dt.float32, tag="pg")
                if NBT > 1:
                    pgb = gpsum.tile([C, 512], mybir.dt.float32, tag="pg")
                pgs = [pga, pgb] if NBT > 1 else [pga]
                for z in range(NZ):
                    pg = pgs[z // ZPB]
                    zo = (z % ZPB) * ZREG
                    nc.tensor.matmul(
                        pg[:, zo:zo + ZREG],
                        shifts[:, z * C:(z + 1) * C],
                        sj[:, soff:soff + ZREG],
                        start=True, stop=True,
                    )
                for z in range(NZ):
                    pg = pgs[z // ZPB]
                    zo = (z % ZPB) * ZREG
                    cpeng = ["vector", "scalar"][(jg + z) % 2] \
                        if geng == "alt" else geng
                    _copy(cpeng,
                          wgp[0:GROWS, woff + z:woff + NGEN * C:NZ],
                          pg[0:GROWS, zo:zo + ZREG])
                return (wgp, woff)

            LOOK = look
            wgs = {}
            for j in range(DPRE):
                issue_w(j)
            for j in range(DPRE, min(DPRE + LOOK, CPC)):
                issue_w(j)
                wgs[j] = gen_tiles(j)

            og = None
            for j in range(CPC):
                nj = j + LOOK + DPRE
                if nj < CPC:
                    issue_w(nj)
                if NGEN and j + LOOK < CPC and j >= DPRE:
                    wgs[j + LOOK] = gen_tiles(j + LOOK)

                wps = wpieces.pop(j)
                if j < DPRE:
                    nd_j = NB
                    wgt = woff = None
                else:
                    nd_j = ND
                    wgt, woff = wgs.pop(j)

                acc = psum.tile([C, NB * B], mybir.dt.float32)
                xt = xts[j // CHX]
                xo = (j % CHX) * NB * B
                for d in range(NB):
                    ncols = B * (NB - d)
                    if d < HS:
                        wsrc = wps[0][:, d * C:(d + 1) * C]
                    elif d < nd_j:
                        wsrc = wps[1][:, (d - HS) * C:(d - HS + 1) * C]
                    else:
                        wsrc = wgt[:, woff + (d - nd_j) * C:
                                   woff + (d - nd_j + 1) * C]
                    nc.tensor.matmul(
                        acc[:, d * B:],
                        wsrc,
                        xt[:, xo:xo + ncols],
                        start=(d == 0),
                        stop=(d == NB - 1),
                    )

                if j % OB == 0:
                    og = opool.tile([C, OB * NB * B], odt, tag="og")
                if oceng == "alt":
                    ceng = ["scalar", "vector"][j % 2]
                elif oceng == "vg":
                    ceng = ["vector", "gpsimd"][j % 2]
                else:
                    ceng = oceng
                _copy(ceng,
                      og[:, (j % OB) * NB * B:(j % OB + 1) * NB * B],
                      acc[:])
                if j >= CPC - OB * obtail and OB > 1:
                    # tail channels: flush per channel to shrink the tail
                    getattr(nc, oeng).dma_start(
                        o_d[:, j * NB * B:(j + 1) * NB * B],
                        og[:, (j % OB) * NB * B:(j % OB + 1) * NB * B])
                elif j % OB == OB - 1:
                    j0 = j - (OB - 1)
                    getattr(nc, oeng).dma_start(
                        o_d[:, j0 * NB * B:(j0 + OB) * NB * B], og[:])

    nc.compile()
    return nc


def _prep_inputs(P, g, ngen=None, nz=None, dpre=None):
    """Host-side shard + layout prep. Returns in_maps list for 8 cores."""
    if ngen is None:
        ngen = KCFG.get("ngen", 4)
    if nz is None:
        nz = KCFG.get("nz", 4)
    if dpre is None:
        dpre = KCFG.get("dpre", 4)
    xsplit = KCFG.get("XSPLIT", 8)
    bf16 = ml_dtypes.bfloat16
    P = np.asarray(P)
    g = np.asarray(g)
    NZ = nz
    SEEDY = C // NZ
    NGEN = ngen
    ND = NB - NGEN
    DPRE = dpre if NGEN else 0

    gmod = g.astype(np.float32).copy()
    gmod[0, :] += 1.0

    if NGEN:
        shifts = np.zeros((C, NZ * C), dtype=bf16)
        for z in range(NZ):
            shifts[:, z * C:(z + 1) * C] = np.eye(C, k=-z, dtype=np.float32)

    in_maps = []
    for core in range(N_CORES):
        lo, hi = core * CPC, (core + 1) * CPC
        gpads = np.zeros((CPC, GLEN), dtype=np.float32)
        gpads[:, 127:127 + T] = gmod[:, lo:hi].T
        gpads = gpads.astype(bf16)

        # full dense Toeplitz expansion: wfull[j, p, e] = gpads[j, e+p]
        sw = np.lib.stride_tricks.sliding_window_view(gpads, NB * C, axis=1)
        wfull = np.ascontiguousarray(sw[:, :C, :])   # (CPC, C, NB*C)

        Pc = P[:, :, lo:hi]                                  # (B, T, CPC)
        x4 = Pc.reshape(B, NB, C, CPC)                       # (b, i, c, j)
        xmov = np.ascontiguousarray(
            x4[:, :, ::-1, :].transpose(2, 3, 1, 0)          # (p, j, i, b)
        ).reshape(C, COLS).astype(bf16)
        xs = xmov.reshape(C, xsplit, COLS // xsplit).transpose(1, 0, 2)
        xs = np.ascontiguousarray(xs)

        m = {"xmov": xs}
        if DPRE:
            m["wpre"] = np.ascontiguousarray(wfull[:DPRE])
        if NGEN:
            NGC = CPC - DPRE
            # seeds[j, p, dd*SEEDY + y] = gpads[j, (ND+dd)*C + p + NZ*y]
            seeds = np.empty((NGC, C, NGEN * SEEDY), dtype=bf16)
            strips = np.empty((NGC, NZ - 1, NGEN * C), dtype=bf16)
            pidx = np.arange(C)[:, None]
            yidx = np.arange(SEEDY)[None, :]
            sidx = np.arange(NZ - 1)[:, None]
            cidx = np.arange(C)[None, :]
            gp = gpads[DPRE:]
            for dd in range(NGEN):
                d = ND + dd
                seeds[:, :, dd * SEEDY:(dd + 1) * SEEDY] = \
                    gp[:, d * C + pidx + NZ * yidx]
                strips[:, :, dd * C:(dd + 1) * C] = \
                    gp[:, d * C + (C - (NZ - 1) + sidx) + cidx]
            m["wdense"] = np.ascontiguousarray(
                np.concatenate([wfull[DPRE:, :, :ND * C], seeds], axis=2))
            m["strips"] = np.ascontiguousarray(
                strips.reshape(NGC // 2, 2, NZ - 1, NGEN * C)
                .transpose(0, 2, 1, 3).reshape(NGC // 2, NZ - 1,
                                               2 * NGEN * C))
            m["shifts"] = shifts
        else:
            m["wdense"] = wfull
        in_maps.append(m)
    return in_maps


def _unshard(results):
    out = np.empty((B, T, NR), np.float32)
    for core in range(N_CORES):
        oc = np.asarray(results[core]["out"], dtype=np.float32)
        oc = oc.reshape(C, CPC, NB, B).transpose(3, 2, 0, 1)  # (b, i, a, j)
        out[:, :, core * CPC:(core + 1) * CPC] = oc.reshape(B, T, CPC)
    return out


KCFG = dict(OB=2, XSPLIT=8, wbufs=10, obf16=True, ngen=4, nz=4,
            gbufs=3, gpbufs=3, geng="vector", sqeng="gpsimd", look=3,
            dpre=4, warmup=0, xlead=0, obtail=1, pbufs=5, obufs=6,
            oeng="sync", xeng="sync", oceng="alt")


def kernel(P, g):
    from concourse.bass_utils import run_bass_kernel_spmd

    if "nc" not in _cache:
        _cache["nc"] = _build_nc(**KCFG)
    nc = _cache["nc"]

    in_maps = _prep_inputs(P, g)
    res = run_bass_kernel_spmd(nc, in_maps, list(range(N_CORES)))
    return _unshard(res.results)


if __name__ == "__main__":
    rng = np.random.default_rng(0)
    P = rng.standard_normal((B, T, NR)).astype(np.float32)
    g = (rng.standard_normal((T, NR)) * 0.1).astype(np.float32)
    out = kernel(P, g)
    print("out shape:", out.shape, out.dtype)


# revision 3
# speedup vs baseline: 1.1645x; 1.1645x over previous
"""Trainium2 Bass kernel for nn_EpsiLayer: per-channel causal full-length
time convolution  out[b,t,j] = P[b,t,j] + sum_{k<=t} g[k,j] * P[b,t-k,j].

Identity fold: with g'[0] = g[0] + 1, out = causal_conv(g', P) exactly.

Per channel j the conv is a lower-triangular Toeplitz (T x T) matmul.
Blocked into C=128 chunks: y_i += W_d @ x_{i-d},
W_d[p, a] = gpad[d*128 + a + p], gpad = 127 zeros ++ g' (bf16); the
moving operand is time-reversed within each block on the host so the
contraction pairs line up.  Each W_d is a 128x128 Hankel slice of the
dense sliding window wdense[p, e] = gpad[e + p].

Sharding: channel-parallel, NR=256 -> 32 channels per core, all B and T
per core, zero inter-core communication.

The kernel streams ~30 MB/core of Toeplitz-expanded weights from HBM
(bf16; fp8/int8 quantization fails the 2e-2 error budget, and Bass has
no int8 matmul).  The tail NGEN=8 offsets per channel are instead
PE-generated from compact seeds via 4 shift-matrix matmuls (seeds are
1/4 the dense bytes), with DVE/ACT copying PSUM->SBUF one pair ahead.

Measured HW cost model (microbenchmarked this session):
  - per-matmul ~ 30 ns fixed + 0.41 ns/col, INDEPENDENT of weight
    reuse, FWL, or explicit ldweights (the embedded weight load is
    already overlapped by the PE reorder window; the fixed cost is
    structural).  Conv floor = 32ch x (32 MM x 30ns + 4224 cols x
    0.41ns) ~ 86 us; HBM stream ~ 30.5 MB / 358 GB/s ~ 85-90 us.
    The kernel sits at this double roofline: ~113-116 us typical,
    with +-5-15 us run-to-run variance from shared-HBM environment.
Alternatives measured SLOWER this session: wide-N gen matmuls with
DVE-side interleave (+5 us), finer weight-DMA piece splitting (+25 us),
PE warmup matmuls (+2 us), x-DMA leading the weight stream (+4 us),
channel-count rebalances of the gen fraction (ngen 4/6 worse by 3-8 us
in matched A/B).  Moving the out-DMA issue queue or splitting the
final out flush measured neutral.
"""

import sys
import numpy as np

try:
    from concourse import bacc, tile  # noqa: F401
except ImportError:
    sys.path.insert(0, "/opt/trn_rl_repo")

import ml_dtypes

B, T, NR = 8, 4096, 256
C = 128
NB = T // C
N_CORES = 8
CPC = NR // N_CORES
COLS = CPC * NB * B
GLEN = 127 + T + 1

_cache = {}


def _build_nc(reps=1, OB=2, XSPLIT=8, oeng="scalar", wbufs=8,
              pbufs=3, obufs=4, ceng="scalar", obf16=True, warmup=30,
              gend0=25, gbufs=3, gpbufs=2, geng="vector", nz=4,
              sqeng="gpsimd", look=2, walt=False, fullgen=3, gwbufs=4,
              tailsplit=0):
    from concourse import bacc, tile
    import concourse.mybir as mybir

    NZ = nz
    SEEDY = C // NZ
    GROWS = C - (NZ - 1)
    NGEN = NB - gend0 if gend0 is not None else 0
    ND = NB - NGEN
    nc = bacc.Bacc("TRN2", target_bir_lowering=False, debug=False)

    FG = fullgen if NGEN else 0
    WCOLS = ND * C + (NGEN * SEEDY if NGEN else 0)
    w_d = nc.dram_tensor("wdense", [CPC - FG, C, WCOLS], mybir.dt.bfloat16,
                         kind="ExternalInput")
    if FG:
        fseed_d = nc.dram_tensor("fseeds", [FG, C, NB * SEEDY],
                                 mybir.dt.bfloat16, kind="ExternalInput")
        fstrip_d = nc.dram_tensor("fstrips", [FG, NZ - 1, NB * C],
                                  mybir.dt.bfloat16, kind="ExternalInput")
    x_d = nc.dram_tensor("xmov", [C, COLS], mybir.dt.bfloat16,
                         kind="ExternalInput")
    if NGEN:
        strip_d = nc.dram_tensor("strips", [CPC // 2, NZ - 1, 2 * NGEN * C],
                                 mybir.dt.bfloat16, kind="ExternalInput")
        shift_d = nc.dram_tensor("shifts", [C, NZ * C], mybir.dt.bfloat16,
                                 kind="ExternalInput")
    odt = mybir.dt.bfloat16 if obf16 else mybir.dt.float32
    o_d = nc.dram_tensor("out", [C, COLS], odt, kind="ExternalOutput")

    def _copy(eng, dst, src):
        if eng == "scalar":
            nc.scalar.activation(dst, src, mybir.ActivationFunctionType.Copy)
        else:
            getattr(nc, eng).tensor_copy(dst, src)

    with tile.TileContext(nc) as tc:
        with (
            tc.tile_pool(name="xpool", bufs=1) as xpool,
            tc.tile_pool(name="wpool", bufs=wbufs) as wpool,
            tc.tile_pool(name="opool", bufs=obufs) as opool,
            tc.tile_pool(name="spool", bufs=4) as spool,
            tc.tile_pool(name="gwpool", bufs=gwbufs) as gwpool,
            tc.tile_pool(name="psum", bufs=pbufs, space="PSUM") as psum,
            tc.tile_pool(name="gpsum", bufs=gpbufs, space="PSUM") as gpsum,
            tc.tile_pool(name="wupsum", bufs=1, space="PSUM") as wupsum,
        ):
            xmov = xpool.tile([C, COLS], mybir.dt.bfloat16)
            XCH = COLS // XSPLIT
            for k in range(XSPLIT):
                nc.scalar.dma_start(xmov[:, k * XCH:(k + 1) * XCH],
                                    x_d[:, k * XCH:(k + 1) * XCH])

            if NGEN:
                shifts = xpool.tile([C, NZ * C], mybir.dt.bfloat16,
                                    tag="shifts")
                nc.sync.dma_start(shifts[:], shift_d.ap())

            if warmup:
                wu = xpool.tile([C, C], mybir.dt.bfloat16, tag="warm")
                wups = wupsum.tile([C, C], mybir.dt.float32, tag="warmp")
                nc.vector.memset(wu[:], 0)
                for _ in range(warmup):
                    nc.tensor.matmul(wups[:], wu[:], wu[:], start=True,
                                     stop=True)

            GCH = 4
            def fullgen_ch(j):
                sj = spool.tile([C, NB * SEEDY], mybir.dt.bfloat16,
                                tag="fseed")
                nc.gpsimd.dma_start(sj[:], fseed_d[j])
                fwg = gwpool.tile([C, NB * C], mybir.dt.bfloat16, tag="fwg")
                nc.gpsimd.dma_start(fwg[GROWS:C, :], fstrip_d[j])
                chunks = [(c0, min(c0 + GCH, NB)) for c0 in range(0, NB, GCH)]
                for ci in range(0, len(chunks), 2):
                    grp = chunks[ci:ci + 2]
                    pgs = [gpsum.tile([C, (c1 - c0) * C], mybir.dt.float32,
                                      tag="pg", name=f"fpg{j}_{c0}")
                           for c0, c1 in grp]
                    for z in range(NZ):
                        for (c0, c1), pg in zip(grp, pgs):
                            nc.tensor.matmul(
                                pg[:, z::NZ],
                                shifts[:, z * C:(z + 1) * C],
                                sj[:, c0 * SEEDY:c1 * SEEDY],
                                start=(z == 0),
                                stop=(z == NZ - 1),
                            )
                    for k, ((c0, c1), pg) in enumerate(zip(grp, pgs)):
                        _copy(["vector", "scalar"][(ci + k) % 2],
                              fwg[0:GROWS, c0 * C:c1 * C], pg[0:GROWS, :])
                return fwg

            pair_tiles = {}
            def gen_tiles(j):
                cpeng = ["vector", "scalar"][j % 2] if geng == "alt" else geng
                wj = wpool.tile([C, WCOLS], mybir.dt.bfloat16)
                weng = [nc.sync, nc.scalar][j % 2] if walt else nc.sync
                weng.dma_start(wj[:], w_d[j - FG])
                sj = wj
                soff = ND * C
                if j % 2 == 0:
                    wgp = gwpool.tile([C, 2 * NGEN * C], mybir.dt.bfloat16,
                                      tag="wg")
                    getattr(nc, sqeng).dma_start(wgp[GROWS:C, :], strip_d[j // 2])
                    pair_tiles[j + 1] = wgp
                else:
                    wgp = pair_tiles.pop(j)
                woff = (j % 2) * NGEN * C
                chunks = [(c0, min(c0 + GCH, NGEN)) for c0 in range(0, NGEN, GCH)]
                pgs = [gpsum.tile([C, (c1 - c0) * C], mybir.dt.float32,
                                  tag="pg", name=f"pg{j}_{c0}")
                       for c0, c1 in chunks]
                for z in range(NZ):
                    for (c0, c1), pg in zip(chunks, pgs):
                        nc.tensor.matmul(
                            pg[:, z::NZ],
                            shifts[:, z * C:(z + 1) * C],
                            sj[:, soff + c0 * SEEDY:soff + c1 * SEEDY],
                            start=(z == 0),
                            stop=(z == NZ - 1),
                        )
                for (c0, c1), pg in zip(chunks, pgs):
                    _copy(cpeng, wgp[0:GROWS, woff + c0 * C:woff + c1 * C],
                          pg[0:GROWS, :])
                return (wj, wgp, woff)

            LOOK = look
            def body(_iv=None):
                wgs = {}
                fwgs = {}
                for j in range(FG):
                    fwgs[j] = fullgen_ch(j)
                if NGEN:
                    for j in range(FG, FG + LOOK):
                        wgs[j] = gen_tiles(j)
                for j in range(CPC):
                    if NGEN and j + LOOK < CPC and j >= FG:
                        wgs[j + LOOK] = gen_tiles(j + LOOK)
                    if j < FG:
                        fwg = fwgs.pop(j)
                        wj = wgt = None
                        woff = 0
                    elif NGEN:
                        wj, wgt, woff = wgs.pop(j)
                        fwg = None
                    else:
                        wj = wpool.tile([C, ND * C], mybir.dt.bfloat16)
                        nc.sync.dma_start(wj[:], w_d[j])
                        fwg = None

                    acc = psum.tile([C, NB * B], mybir.dt.float32)
                    xj = xmov[:, j * NB * B:(j + 1) * NB * B]
                    for d in range(NB):
                        ncols = B * (NB - d)
                        if fwg is not None:
                            wsrc = fwg[:, d * C:(d + 1) * C]
                        elif d < ND:
                            wsrc = wj[:, d * C:(d + 1) * C]
                        else:
                            wsrc = wgt[:, woff + (d - ND) * C:
                                       woff + (d - ND + 1) * C]
                        nc.tensor.matmul(
                            acc[:, d * B:],
                            wsrc,
                            xj[:, :ncols],
                            start=(d == 0),
                            stop=(d == NB - 1),
                        )

                    if j % OB == 0:
                        og = opool.tile([C, OB * NB * B], odt, tag="og")
                    _copy(ceng,
                          og[:, (j % OB) * NB * B:(j % OB + 1) * NB * B],
                          acc[:])
                    if j % OB == OB - 1:
                        j0 = j - (OB - 1)
                        if tailsplit and j == CPC - 1:
                            for h in range(OB):
                                getattr(nc, oeng).dma_start(
                                    o_d[:, (j0 + h) * NB * B:
                                        (j0 + h + 1) * NB * B],
                                    og[:, h * NB * B:(h + 1) * NB * B])
                        else:
                            getattr(nc, oeng).dma_start(
                                o_d[:, j0 * NB * B:(j0 + OB) * NB * B],
                                og[:])

            if reps == 1:
                body()
            else:
                with tc.For_i(0, reps, 1) as iv:
                    body(iv)

    nc.compile()
    return nc


def _prep_inputs(P, g, gend0=None, nz=None, fullgen=None):
    if gend0 is None:
        gend0 = KCFG.get("gend0")
    if nz is None:
        nz = KCFG.get("nz", 4)
    if fullgen is None:
        fullgen = KCFG.get("fullgen", 0)
    bf16 = ml_dtypes.bfloat16
    P = np.asarray(P)
    g = np.asarray(g)
    NZ = nz
    SEEDY = C // NZ
    NGEN = NB - gend0 if gend0 is not None else 0
    ND = NB - NGEN
    FG = fullgen if NGEN else 0

    gmod = g.astype(np.float32).copy()
    gmod[0, :] += 1.0

    if NGEN:
        shifts = np.zeros((C, NZ * C), dtype=bf16)
        for z in range(NZ):
            shifts[:, z * C:(z + 1) * C] = np.eye(C, k=-z, dtype=np.float32)

    in_maps = []
    for core in range(N_CORES):
        lo, hi = core * CPC, (core + 1) * CPC
        gpads = np.zeros((CPC, GLEN), dtype=np.float32)
        gpads[:, 127:127 + T] = gmod[:, lo:hi].T
        gpads = gpads.astype(bf16)

        sw = np.lib.stride_tricks.sliding_window_view(gpads, ND * C, axis=1)
        wdense = np.ascontiguousarray(sw[:, :C, :])

        Pc = P[:, :, lo:hi]
        x4 = Pc.reshape(B, NB, C, CPC)
        xmov = np.ascontiguousarray(
            x4[:, :, ::-1, :].transpose(2, 3, 1, 0)
        ).reshape(C, COLS).astype(bf16)

        m = {"xmov": xmov, "wdense": wdense}
        if NGEN:
            seeds = np.empty((CPC, C, NGEN * SEEDY), dtype=bf16)
            strips = np.empty((CPC, NZ - 1, NGEN * C), dtype=bf16)
            pidx = np.arange(C)[:, None]
            yidx = np.arange(SEEDY)[None, :]
            sidx = np.arange(NZ - 1)[:, None]
            cidx = np.arange(C)[None, :]
            for dd in range(NGEN):
                d = ND + dd
                seeds[:, :, dd * SEEDY:(dd + 1) * SEEDY] = \
                    gpads[:, d * C + pidx + NZ * yidx]
                strips[:, :, dd * C:(dd + 1) * C] = \
                    gpads[:, d * C + (C - (NZ - 1) + sidx) + cidx]
            m["wdense"] = np.ascontiguousarray(
                np.concatenate([wdense, seeds], axis=2)[FG:])
            m["strips"] = np.ascontiguousarray(
                strips.reshape(CPC // 2, 2, NZ - 1, NGEN * C)
                .transpose(0, 2, 1, 3).reshape(CPC // 2, NZ - 1, 2 * NGEN * C))
            m["shifts"] = shifts
            if FG:
                fseeds = np.empty((FG, C, NB * SEEDY), dtype=bf16)
                fstrips = np.empty((FG, NZ - 1, NB * C), dtype=bf16)
                for dd in range(NB):
                    fseeds[:, :, dd * SEEDY:(dd + 1) * SEEDY] = \
                        gpads[:FG, dd * C + pidx + NZ * yidx]
                    fstrips[:, :, dd * C:(dd + 1) * C] = \
                        gpads[:FG, dd * C + (C - (NZ - 1) + sidx) + cidx]
                m["fseeds"] = fseeds
                m["fstrips"] = fstrips
        in_maps.append(m)
    return in_maps


def _unshard(results):
    out = np.empty((B, T, NR), np.float32)
    for core in range(N_CORES):
        oc = np.asarray(results[core]["out"], dtype=np.float32)
        oc = oc.reshape(C, CPC, NB, B).transpose(3, 2, 0, 1)
        out[:, :, core * CPC:(core + 1) * CPC] = oc.reshape(B, T, CPC)
    return out


KCFG = dict(OB=2, XSPLIT=4, wbufs=16, obf16=True, warmup=0, gend0=24, nz=4,
            pbufs=4, gpbufs=4, geng="alt", sqeng="gpsimd", look=4, fullgen=0)


def kernel(P, g):
    from concourse.bass_utils import run_bass_kernel_spmd

    if "nc" not in _cache:
        _cache["nc"] = _build_nc(**KCFG)
    nc = _cache["nc"]

    in_maps = _prep_inputs(P, g, gend0=KCFG.get("gend0"),
                           nz=KCFG.get("nz", 4),
                           fullgen=KCFG.get("fullgen", 0))
    res = run_bass_kernel_spmd(nc, in_maps, list(range(N_CORES)))
    return _unshard(res.results)


# revision 4
# speedup vs baseline: 1.1700x; 1.0047x over previous
"""Trainium2 Bass kernel for nn_EpsiLayer: per-channel causal full-length
time convolution  out[b,t,j] = P[b,t,j] + sum_{k<=t} g[k,j] * P[b,t-k,j].

Identity fold: with g'[0] = g[0] + 1, out = causal_conv(g', P) exactly.
Per channel j the conv is a lower-triangular Toeplitz (T x T) matmul,
blocked into C=128 chunks: y_i += W_d @ x_{i-d} with Hankel tiles
W_d[p, a] = gpad[d*128 + a + p], gpad = 127 zeros ++ g' (bf16).
Sharding: channel-parallel, 32 channels/core on 8 cores, no comms.

The kernel sits at a measured double roofline (~113-116 us wall,
+-5-15 us shared-HBM environment variance):
  - PE: per-matmul ~ 30 ns fixed + 0.41 ns/col (microbenchmarked;
    INDEPENDENT of weight reuse / FWL / explicit ldweights - the
    embedded weight load is already hidden by the PE reorder window).
    Conv = 32ch x 32 MMs, avg N=132 -> ~86 us; tail-tile generation
    (below) adds ~21 us -> PE busy ~107 us.
  - DMA: ~30.5 MB/core weight+x+out stream at ~300-358 GB/s ~ 95-101
    us.  gend0=24 (8 of 32 tiles PE-generated from seeds at 1/4 the
    dense bytes) balances the two; raising or lowering NGEN measured
    worse in matched A/B.
Tail offsets d>=24 are generated on-PE: seeds S[p,y]=gpad[d*128+p+4y]
ship packed in the weight DMA; 4 shift-matrix matmuls (z=0..3) scatter
them into PSUM columns z::4 (z-outer over 2 chunks so LDWEIGHTS hides
under the same-Sigma stream); DVE/ACT copy PSUM->SBUF one pair ahead;
bottom 3 partitions (wraparound rows) come from a tiny HBM strip (a
circular-shift stationary cannot replace them: the wrap needs the
NEXT tile's seed column).  x is loaded as XSPLIT separate tiles so
early channels depend only on their own x slice.

Measured SLOWER or neutral this session (kept out): wide-N gen MMs
with contiguous per-z PSUM + DVE stride-interleave (+5 us), finer
weight-DMA piece splitting (+25 us), PE warmup MMs (+2 us), x-DMA
leading the weight stream (+4 us), ngen 4/6/dense-prologue variants
(+3-8 us), out-DMA on the sync ring (neutral), final-flush split
(neutral), fp8/int8 weights (fail 2e-2 error budget / no int8 matmul
in Bass).
"""

import sys
import numpy as np

try:
    from concourse import bacc, tile  # noqa: F401
except ImportError:
    sys.path.insert(0, "/opt/trn_rl_repo")

import ml_dtypes

B, T, NR = 8, 4096, 256
C = 128
NB = T // C
N_CORES = 8
CPC = NR // N_CORES
COLS = CPC * NB * B
GLEN = 127 + T + 1

_cache = {}


def _build_nc(reps=1, OB=2, XSPLIT=8, oeng="scalar", wbufs=8,
              pbufs=3, obufs=4, ceng="scalar", obf16=True, warmup=30,
              gend0=25, gbufs=3, gpbufs=2, geng="vector", nz=4,
              sqeng="gpsimd", look=2, walt=False, fullgen=3, gwbufs=4,
              tailsplit=0, xtiles=0):
    from concourse import bacc, tile
    import concourse.mybir as mybir

    NZ = nz
    SEEDY = C // NZ
    GROWS = C - (NZ - 1)
    NGEN = NB - gend0 if gend0 is not None else 0
    ND = NB - NGEN
    nc = bacc.Bacc("TRN2", target_bir_lowering=False, debug=False)

    FG = fullgen if NGEN else 0
    WCOLS = ND * C + (NGEN * SEEDY if NGEN else 0)
    w_d = nc.dram_tensor("wdense", [CPC - FG, C, WCOLS], mybir.dt.bfloat16,
                         kind="ExternalInput")
    if FG:
        fseed_d = nc.dram_tensor("fseeds", [FG, C, NB * SEEDY],
                                 mybir.dt.bfloat16, kind="ExternalInput")
        fstrip_d = nc.dram_tensor("fstrips", [FG, NZ - 1, NB * C],
                                  mybir.dt.bfloat16, kind="ExternalInput")
    x_d = nc.dram_tensor("xmov", [C, COLS], mybir.dt.bfloat16,
                         kind="ExternalInput")
    if NGEN:
        strip_d = nc.dram_tensor("strips", [CPC // 2, NZ - 1, 2 * NGEN * C],
                                 mybir.dt.bfloat16, kind="ExternalInput")
        shift_d = nc.dram_tensor("shifts", [C, NZ * C], mybir.dt.bfloat16,
                                 kind="ExternalInput")
    odt = mybir.dt.bfloat16 if obf16 else mybir.dt.float32
    o_d = nc.dram_tensor("out", [C, COLS], odt, kind="ExternalOutput")

    def _copy(eng, dst, src):
        if eng == "scalar":
            nc.scalar.activation(dst, src, mybir.ActivationFunctionType.Copy)
        else:
            getattr(nc, eng).tensor_copy(dst, src)

    with tile.TileContext(nc) as tc:
        with (
            tc.tile_pool(name="xpool", bufs=1) as xpool,
            tc.tile_pool(name="wpool", bufs=wbufs) as wpool,
            tc.tile_pool(name="opool", bufs=obufs) as opool,
            tc.tile_pool(name="spool", bufs=4) as spool,
            tc.tile_pool(name="gwpool", bufs=gwbufs) as gwpool,
            tc.tile_pool(name="psum", bufs=pbufs, space="PSUM") as psum,
            tc.tile_pool(name="gpsum", bufs=gpbufs, space="PSUM") as gpsum,
            tc.tile_pool(name="wupsum", bufs=1, space="PSUM") as wupsum,
        ):
            XCH = COLS // XSPLIT
            if xtiles:
                assert XCH % (NB * B) == 0
                xts = [xpool.tile([C, XCH], mybir.dt.bfloat16,
                                  tag=f"x{k}", name=f"x{k}")
                       for k in range(XSPLIT)]
                for k in range(XSPLIT):
                    nc.scalar.dma_start(xts[k][:],
                                        x_d[:, k * XCH:(k + 1) * XCH])
            else:
                xmov = xpool.tile([C, COLS], mybir.dt.bfloat16)
                for k in range(XSPLIT):
                    nc.scalar.dma_start(xmov[:, k * XCH:(k + 1) * XCH],
                                        x_d[:, k * XCH:(k + 1) * XCH])

            if NGEN:
                shifts = xpool.tile([C, NZ * C], mybir.dt.bfloat16,
                                    tag="shifts")
                nc.sync.dma_start(shifts[:], shift_d.ap())

            if warmup:
                wu = xpool.tile([C, C], mybir.dt.bfloat16, tag="warm")
                wups = wupsum.tile([C, C], mybir.dt.float32, tag="warmp")
                nc.vector.memset(wu[:], 0)
                for _ in range(warmup):
                    nc.tensor.matmul(wups[:], wu[:], wu[:], start=True,
                                     stop=True)

            GCH = 4
            def fullgen_ch(j):
                sj = spool.tile([C, NB * SEEDY], mybir.dt.bfloat16,
                                tag="fseed")
                nc.gpsimd.dma_start(sj[:], fseed_d[j])
                fwg = gwpool.tile([C, NB * C], mybir.dt.bfloat16, tag="fwg")
                nc.gpsimd.dma_start(fwg[GROWS:C, :], fstrip_d[j])
                chunks = [(c0, min(c0 + GCH, NB)) for c0 in range(0, NB, GCH)]
                for ci in range(0, len(chunks), 2):
                    grp = chunks[ci:ci + 2]
                    pgs = [gpsum.tile([C, (c1 - c0) * C], mybir.dt.float32,
                                      tag="pg", name=f"fpg{j}_{c0}")
                           for c0, c1 in grp]
                    for z in range(NZ):
                        for (c0, c1), pg in zip(grp, pgs):
                            nc.tensor.matmul(
                                pg[:, z::NZ],
                                shifts[:, z * C:(z + 1) * C],
                                sj[:, c0 * SEEDY:c1 * SEEDY],
                                start=(z == 0),
                                stop=(z == NZ - 1),
                            )
                    for k, ((c0, c1), pg) in enumerate(zip(grp, pgs)):
                        _copy(["vector", "scalar"][(ci + k) % 2],
                              fwg[0:GROWS, c0 * C:c1 * C], pg[0:GROWS, :])
                return fwg

            pair_tiles = {}
            def gen_tiles(j):
                cpeng = ["vector", "scalar"][j % 2] if geng == "alt" else geng
                wj = wpool.tile([C, WCOLS], mybir.dt.bfloat16)
                weng = [nc.sync, nc.scalar][j % 2] if walt else nc.sync
                weng.dma_start(wj[:], w_d[j - FG])
                sj = wj
                soff = ND * C
                if j % 2 == 0:
                    wgp = gwpool.tile([C, 2 * NGEN * C], mybir.dt.bfloat16,
                                      tag="wg")
                    getattr(nc, sqeng).dma_start(wgp[GROWS:C, :], strip_d[j // 2])
                    pair_tiles[j + 1] = wgp
                else:
                    wgp = pair_tiles.pop(j)
                woff = (j % 2) * NGEN * C
                chunks = [(c0, min(c0 + GCH, NGEN)) for c0 in range(0, NGEN, GCH)]
                pgs = [gpsum.tile([C, (c1 - c0) * C], mybir.dt.float32,
                                  tag="pg", name=f"pg{j}_{c0}")
                       for c0, c1 in chunks]
                for z in range(NZ):
                    for (c0, c1), pg in zip(chunks, pgs):
                        nc.tensor.matmul(
                            pg[:, z::NZ],
                            shifts[:, z * C:(z + 1) * C],
                            sj[:, soff + c0 * SEEDY:soff + c1 * SEEDY],
                            start=(z == 0),
                            stop=(z == NZ - 1),
                        )
                for (c0, c1), pg in zip(chunks, pgs):
                    _copy(cpeng, wgp[0:GROWS, woff + c0 * C:woff + c1 * C],
                          pg[0:GROWS, :])
                return (wj, wgp, woff)

            LOOK = look
            def body(_iv=None):
                wgs = {}
                fwgs = {}
                for j in range(FG):
                    fwgs[j] = fullgen_ch(j)
                if NGEN:
                    for j in range(FG, FG + LOOK):
                        wgs[j] = gen_tiles(j)
                for j in range(CPC):
                    if NGEN and j + LOOK < CPC and j >= FG:
                        wgs[j + LOOK] = gen_tiles(j + LOOK)
                    if j < FG:
                        fwg = fwgs.pop(j)
                        wj = wgt = None
                        woff = 0
                    elif NGEN:
                        wj, wgt, woff = wgs.pop(j)
                        fwg = None
                    else:
                        wj = wpool.tile([C, ND * C], mybir.dt.bfloat16)
                        nc.sync.dma_start(wj[:], w_d[j])
                        fwg = None

                    acc = psum.tile([C, NB * B], mybir.dt.float32)
                    if xtiles:
                        xo = j * NB * B
                        xj = xts[xo // XCH][:, xo % XCH:xo % XCH + NB * B]
                    else:
                        xj = xmov[:, j * NB * B:(j + 1) * NB * B]
                    for d in range(NB):
                        ncols = B * (NB - d)
                        if fwg is not None:
                            wsrc = fwg[:, d * C:(d + 1) * C]
                        elif d < ND:
                            wsrc = wj[:, d * C:(d + 1) * C]
                        else:
                            wsrc = wgt[:, woff + (d - ND) * C:
                                       woff + (d - ND + 1) * C]
                        nc.tensor.matmul(
                            acc[:, d * B:],
                            wsrc,
                            xj[:, :ncols],
                            start=(d == 0),
                            stop=(d == NB - 1),
                        )

                    if j % OB == 0:
                        og = opool.tile([C, OB * NB * B], odt, tag="og")
                    _copy(ceng,
                          og[:, (j % OB) * NB * B:(j % OB + 1) * NB * B],
                          acc[:])
                    if j % OB == OB - 1:
                        j0 = j - (OB - 1)
                        if tailsplit and j == CPC - 1:
                            for h in range(OB):
                                getattr(nc, oeng).dma_start(
                                    o_d[:, (j0 + h) * NB * B:
                                        (j0 + h + 1) * NB * B],
                                    og[:, h * NB * B:(h + 1) * NB * B])
                        else:
                            getattr(nc, oeng).dma_start(
                                o_d[:, j0 * NB * B:(j0 + OB) * NB * B],
                                og[:])

            if reps == 1:
                body()
            else:
                with tc.For_i(0, reps, 1) as iv:
                    body(iv)

    nc.compile()
    return nc


def _prep_inputs(P, g, gend0=None, nz=None, fullgen=None):
    if gend0 is None:
        gend0 = KCFG.get("gend0")
    if nz is None:
        nz = KCFG.get("nz", 4)
    if fullgen is None:
        fullgen = KCFG.get("fullgen", 0)
    bf16 = ml_dtypes.bfloat16
    P = np.asarray(P)
    g = np.asarray(g)
    NZ = nz
    SEEDY = C // NZ
    NGEN = NB - gend0 if gend0 is not None else 0
    ND = NB - NGEN
    FG = fullgen if NGEN else 0

    gmod = g.astype(np.float32).copy()
    gmod[0, :] += 1.0

    if NGEN:
        shifts = np.zeros((C, NZ * C), dtype=bf16)
        for z in range(NZ):
            shifts[:, z * C:(z + 1) * C] = np.eye(C, k=-z, dtype=np.float32)

    in_maps = []
    for core in range(N_CORES):
        lo, hi = core * CPC, (core + 1) * CPC
        gpads = np.zeros((CPC, GLEN), dtype=np.float32)
        gpads[:, 127:127 + T] = gmod[:, lo:hi].T
        gpads = gpads.astype(bf16)

        sw = np.lib.stride_tricks.sliding_window_view(gpads, ND * C, axis=1)
        wdense = np.ascontiguousarray(sw[:, :C, :])

        Pc = P[:, :, lo:hi]
        x4 = Pc.reshape(B, NB, C, CPC)
        xmov = np.ascontiguousarray(
            x4[:, :, ::-1, :].transpose(2, 3, 1, 0)
        ).reshape(C, COLS).astype(bf16)

        m = {"xmov": xmov, "wdense": wdense}
        if NGEN:
            seeds = np.empty((CPC, C, NGEN * SEEDY), dtype=bf16)
            strips = np.empty((CPC, NZ - 1, NGEN * C), dtype=bf16)
            pidx = np.arange(C)[:, None]
            yidx = np.arange(SEEDY)[None, :]
            sidx = np.arange(NZ - 1)[:, None]
            cidx = np.arange(C)[None, :]
            for dd in range(NGEN):
                d = ND + dd
                seeds[:, :, dd * SEEDY:(dd + 1) * SEEDY] = \
                    gpads[:, d * C + pidx + NZ * yidx]
                strips[:, :, dd * C:(dd + 1) * C] = \
                    gpads[:, d * C + (C - (NZ - 1) + sidx) + cidx]
            m["wdense"] = np.ascontiguousarray(
                np.concatenate([wdense, seeds], axis=2)[FG:])
            m["strips"] = np.ascontiguousarray(
                strips.reshape(CPC // 2, 2, NZ - 1, NGEN * C)
                .transpose(0, 2, 1, 3).reshape(CPC // 2, NZ - 1, 2 * NGEN * C))
            m["shifts"] = shifts
            if FG:
                fseeds = np.empty((FG, C, NB * SEEDY), dtype=bf16)
                fstrips = np.empty((FG, NZ - 1, NB * C), dtype=bf16)
                for dd in range(NB):
                    fseeds[:, :, dd * SEEDY:(dd + 1) * SEEDY] = \
                        gpads[:FG, dd * C + pidx + NZ * yidx]
                    fstrips[:, :, dd * C:(dd + 1) * C] = \
                        gpads[:FG, dd * C + (C - (NZ - 1) + sidx) + cidx]
                m["fseeds"] = fseeds
                m["fstrips"] = fstrips
        in_maps.append(m)
    return in_maps


def _unshard(results):
    out = np.empty((B, T, NR), np.float32)
    for core in range(N_CORES):
        oc = np.asarray(results[core]["out"], dtype=np.float32)
        oc = oc.reshape(C, CPC, NB, B).transpose(3, 2, 0, 1)
        out[:, :, core * CPC:(core + 1) * CPC] = oc.reshape(B, T, CPC)
    return out


KCFG = dict(OB=2, XSPLIT=4, wbufs=16, obf16=True, warmup=0, gend0=24, nz=4,
            pbufs=4, gpbufs=4, geng="alt", sqeng="gpsimd", look=4, fullgen=0,
            xtiles=1)


def kernel(P, g):
    from concourse.bass_utils import run_bass_kernel_spmd

    if "nc" not in _cache:
        _cache["nc"] = _build_nc(**KCFG)
    nc = _cache["nc"]

    in_maps = _prep_inputs(P, g, gend0=KCFG.get("gend0"),
                           nz=KCFG.get("nz", 4),
                           fullgen=KCFG.get("fullgen", 0))
    res = run_bass_kernel_spmd(nc, in_maps, list(range(N_CORES)))
    return _unshard(res.results)
